# revision 2
# baseline (speedup 1.0000x reference)
"""GATv2 x2 + edge decoder (gnn_message_passing) on 8 TRN2 NeuronCores.

Strategy (dst-sharded edge phase):
- Edges (incl. self-loops) are sorted by dst on the host and partitioned into
  per-dst-tile (128 nodes) contiguous runs, padded to a uniform number of
  128-edge subtiles (S_sub) so the SPMD program is identical on every core.
- Core k owns dst nodes [k*N/8, (k+1)*N/8): segment softmax/sum stay local.
- Per layer: xl = x@Wl.T+bl is computed (replicated GEMM) into a DRAM table;
  per edge-subtile xl[src] rows are fetched with indirect DMA (4KB/row).
  xr is never materialized: within a dst tile, xr[dst_e] is expanded from the
  128-node xr tile with a PE matmul against a selection matrix
  sel[e,n] = (dstloc_e == n), which also performs the segment-sum scatter
  (out += sel.T @ msg) and denominator (den += sel.T @ ex) as matmuls.
- Segment softmax skips the segment-max subtraction (mathematically identical;
  scores are O(1) here so exp cannot overflow).
- Between layers, z.T slices are AllGathered so every core can run the
  replicated GEMMs of the next layer.
- Decoder: zc@Wd1.T splits into P[src]+Q[dst] with P = z@Wd1[:, :C].T + bd1,
  Q = z@Wd1[:, C:].T (per-node GEMMs), then per-edge gather/add/lrelu/dot.
"""

import sys

sys.path.insert(0, "/opt/trn_rl_repo")

import numpy as np

import bass_rust
import concourse.bass as bass
import concourse.mybir as mybir
import concourse.tile as tile

P = 128
NS_ATT = 0.2
NS_ACT = 0.01
dt = mybir.dt
Alu = mybir.AluOpType
Act = mybir.ActivationFunctionType


# ---------------------------------------------------------------------------
# workaround: this walrus build rejects sem waits attached to InstDrain
# ("Too many sync wait commands"); hoist every drain wait onto NoOps.
def _fix_waits(nc, max_other=1):
    for bb in nc.main_func.blocks:
        newlist = []
        for ins in bb.instructions:
            si = ins.sync_info
            if si is not None and si.on_wait:
                waits = list(si.on_wait)
                no_wait = isinstance(ins, mybir.InstDrain) or hasattr(ins, "isa_opcode")
                limit = 0 if no_wait else max_other
                if len(waits) > limit:
                    nkeep = limit
                    extra = waits[: len(waits) - nkeep] if nkeep else waits
                    keep = waits[len(waits) - nkeep:] if nkeep else []
                    k = 0
                    while extra:
                        chunk, extra = extra[:1], extra[1:]
                        nop = mybir.InstNoOp(
                            name=f"{ins.name}_ws{k}", engine=ins.engine, ins=[], outs=[]
                        )
                        nop.sync_info = bass_rust.SyncInfo(on_wait=chunk, on_update=[])
                        newlist.append(nop)
                        k += 1
                    ins.sync_info = bass_rust.SyncInfo(
                        on_wait=keep, on_update=list(si.on_update or [])
                    )
            newlist.append(ins)
        bb.instructions = newlist


def _chunks(total, step=512):
    return [(i, min(i + step, total)) for i in range(0, total, step)]


def build_program(cfg, fix=True, dbg=False):
    """Build the SPMD Bass program. cfg keys:
    N, C (=IN=out_c), H, n_cores, S_sub, DSUB (decoder subtiles/core),
    GB (edge gather batch), GBD (decoder gather batch), bd2 (float).
    """
    N, C, H = cfg["N"], cfg["C"], cfg["H"]
    NC_ = cfg["n_cores"]
    HC = H * C
    NPC = N // NC_        # nodes per core
    T = NPC // P          # dst tiles per core
    TN = N // P           # total node tiles
    S = cfg["S_sub"]
    DSUB = cfg["DSUB"]
    GB = cfg["GB"]
    GBD = cfg["GBD"]

    nc = bass.Bass()

    def inp(name, shape, d=dt.float32):
        return nc.declare_dram_parameter(name, list(shape), d, isOutput=False)

    x_T = inp("x_T", [C, N])
    x_own_T = inp("x_own_T", [C, NPC])
    srcT = inp("srcT", [P, T * S], dt.int32)
    dstlocT = inp("dstlocT", [P, T * S])
    dec_srcT = inp("dec_srcT", [P, DSUB], dt.int32)
    dec_dstT = inp("dec_dstT", [P, DSUB], dt.int32)
    wl1T = inp("wl1T", [C, HC])
    wr1T = inp("wr1T", [C, HC])
    wl2T = inp("wl2T", [C, HC])
    wr2T = inp("wr2T", [C, HC])
    blr1 = inp("blr1", [1, HC])
    brr1 = inp("brr1", [1, HC])
    blr2 = inp("blr2", [1, HC])
    brr2 = inp("brr2", [1, HC])
    att1_t = inp("att1_t", [P, HC])
    att2_t = inp("att2_t", [P, HC])
    bias1_t = inp("bias1_t", [P, C])
    bias2_t = inp("bias2_t", [P, C])
    a_t = inp("a_t", [C, C])
    b_t = inp("b_t", [C, C])
    bd1r = inp("bd1r", [1, C])
    wd2_t = inp("wd2_t", [P, C])
    iota_t = inp("iota_t", [P, P])
    ident_t = inp("ident_t", [P, P])
    ones_t = inp("ones_t", [1, P])

    out_dec = nc.declare_dram_parameter("out_dec", [DSUB, P], dt.float32, isOutput=True)
    if dbg:
        dbg_xl1 = nc.declare_dram_parameter("dbg_xl1", [2 * P, HC], dt.float32, isOutput=True)
        dbg_zT = nc.declare_dram_parameter("dbg_zT", [C, NPC], dt.float32, isOutput=True)
        dbg_zall = nc.declare_dram_parameter("dbg_zall", [NC_ * C, NPC], dt.float32, isOutput=True)
        dbg_xl2 = nc.declare_dram_parameter("dbg_xl2", [2 * P, HC], dt.float32, isOutput=True)
        dbg_z2T = nc.declare_dram_parameter("dbg_z2T", [C, NPC], dt.float32, isOutput=True)
        dbg_p = nc.declare_dram_parameter("dbg_p", [2 * P, C], dt.float32, isOutput=True)
        dbg_q = nc.declare_dram_parameter("dbg_q", [2 * P, C], dt.float32, isOutput=True)
        dbg_xr = nc.declare_dram_parameter("dbg_xr", [P, HC], dt.float32, isOutput=True)
        dbg_sel = nc.declare_dram_parameter("dbg_sel", [P, P], dt.float32, isOutput=True)
        dbg_selT = nc.declare_dram_parameter("dbg_selT", [P, P], dt.float32, isOutput=True)
        dbg_lr = nc.declare_dram_parameter("dbg_lr", [P, HC], dt.float32, isOutput=True)
        dbg_s8 = nc.declare_dram_parameter("dbg_s8", [P, H], dt.float32, isOutput=True)
        dbg_ex = nc.declare_dram_parameter("dbg_ex", [P, H], dt.float32, isOutput=True)
        dbg_msg = nc.declare_dram_parameter("dbg_msg", [P, HC], dt.float32, isOutput=True)
        dbg_den = nc.declare_dram_parameter("dbg_den", [P, H], dt.float32, isOutput=True)
        dbg_osb = nc.declare_dram_parameter("dbg_osb", [P, HC], dt.float32, isOutput=True)

    with tile.TileContext(nc) as tc:
        with (
            tc.tile_pool(name="const", bufs=1) as cp,
            tc.tile_pool(name="work", bufs=2) as wp,
            tc.tile_pool(name="gpool", bufs=3) as gp,
            tc.tile_pool(name="psum", bufs=2, space="PSUM") as pp,
            tc.tile_pool(name="psum1", bufs=1, space="PSUM") as pp1,
            tc.tile_pool(name="dram", bufs=1, space="DRAM") as dp,
        ):
            # ---- load constants -------------------------------------------------
            def cload(ap, shape, d=dt.float32):
                t_ = cp.tile(list(shape), d, tag=f"c_{ap.name}")
                nc.sync.dma_start(out=t_[:], in_=ap[:, :])
                return t_

            wl1_sb = cload(wl1T, [C, HC])
            wr1_sb = cload(wr1T, [C, HC])
            wl2_sb = cload(wl2T, [C, HC])
            wr2_sb = cload(wr2T, [C, HC])
            blr1_sb = cload(blr1, [1, HC])
            brr1_sb = cload(brr1, [1, HC])
            blr2_sb = cload(blr2, [1, HC])
            brr2_sb = cload(brr2, [1, HC])
            att1_sb = cload(att1_t, [P, HC])
            att2_sb = cload(att2_t, [P, HC])
            bias1_sb = cload(bias1_t, [P, C])
            bias2_sb = cload(bias2_t, [P, C])
            a_sb = cload(a_t, [C, C])
            b_sb = cload(b_t, [C, C])
            bd1_sb = cload(bd1r, [1, C])
            wd2_sb = cload(wd2_t, [P, C])
            iota_sb = cload(iota_t, [P, P])
            ident_sb = cload(ident_t, [P, P])
            ones_sb = cload(ones_t, [1, P])
            srcT_sb = cload(srcT, [P, T * S], dt.int32)
            dstloc_sb = cload(dstlocT, [P, T * S])
            dsrc_sb = cload(dec_srcT, [P, DSUB], dt.int32)
            ddst_sb = cload(dec_dstT, [P, DSUB], dt.int32)
            xown_sb = cload(x_own_T, [C, NPC])
            res_sb = cp.tile([P, DSUB], dt.float32, tag="res_sb")

            # ---- DRAM scratch ---------------------------------------------------
            xl1_tab = dp.tile([N, HC], dt.float32)
            xl2_tab = dp.tile([N, HC], dt.float32)
            zT_loc = dp.tile([C, NPC], dt.float32)
            zT_all = dp.tile([NC_, C, NPC], dt.float32)
            z2T_loc = dp.tile([C, NPC], dt.float32)
            z2T_all = dp.tile([NC_, C, NPC], dt.float32)
            p_tab = dp.tile([N, C], dt.float32)
            q_tab = dp.tile([N, C], dt.float32)

            def biased_gemm(lhsT_sb, wT_sb, brow_sb, ps, width):
                """ps[P, width] = bias_row + lhsT.T @ wT  (chunked by 512)."""
                for n0, n1 in _chunks(width):
                    nc.tensor.matmul(out=ps[:, n0:n1], lhsT=ones_sb[:, :],
                                     rhs=brow_sb[:, n0:n1], start=True, stop=False)
                    nc.tensor.matmul(out=ps[:, n0:n1], lhsT=lhsT_sb[:],
                                     rhs=wT_sb[:, n0:n1], start=False, stop=True)

            # ---- phase A/D: xl table GEMM (replicated over nodes) ---------------
            def dense_phase(wT_sb, brow_sb, xl_tab, src_of_tile):
                for nt in range(TN):
                    lt = wp.tile([C, P], dt.float32, tag="lhsT")
                    nc.sync.dma_start(out=lt[:], in_=src_of_tile(nt))
                    ps = pp.tile([P, HC], dt.float32, tag="big")
                    biased_gemm(lt, wT_sb, brow_sb, ps, HC)
                    xsb = gp.tile([P, HC], dt.float32, tag="xl_out")
                    nc.scalar.activation(out=xsb[:], in_=ps[:], func=Act.Copy)
                    nc.sync.dma_start(out=xl_tab[nt * P:(nt + 1) * P, :], in_=xsb[:])

            # ---- phase B/E: edge phase (dst-sharded) ----------------------------
            def edge_phase(xl_tab, wrT_sb, brow_sb, att_sb, bias_sb, zT_out, act):
                tap = dbg and xl_tab is xl1_tab
                inv_h = 1.0 / H
                for t in range(T):
                    tap_t = tap and t == 0
                    xo = wp.tile([C, P], dt.float32, tag="lhsT")
                    if xl_tab is xl1_tab:
                        nc.sync.dma_start(out=xo[:], in_=xown_sb[:, t * P:(t + 1) * P])
                    else:
                        nc.sync.dma_start(out=xo[:], in_=zT_loc[:, t * P:(t + 1) * P])
                    psx = pp.tile([P, HC], dt.float32, tag="big")
                    biased_gemm(xo, wrT_sb, brow_sb, psx, HC)
                    xr_sb = wp.tile([P, HC], dt.float32, tag="xr")
                    nc.scalar.activation(out=xr_sb[:], in_=psx[:], func=Act.Copy)
                    if tap_t:
                        nc.sync.dma_start(out=dbg_xr[:, :], in_=xr_sb[:])

                    out_ps = pp1.tile([P, HC], dt.float32, tag="out")
                    den_ps = pp1.tile([P, H], dt.float32, tag="den")

                    for c0 in range(0, S, GB):
                        gb = min(GB, S - c0)
                        gbuf = gp.tile([P, GB * HC], dt.float32, tag="gbuf")
                        col0 = t * S + c0
                        nc.gpsimd.indirect_dma_start(
                            out=gbuf[:, : gb * HC], out_offset=None,
                            in_=xl_tab[:, :],
                            in_offset=bass.IndirectOffsetOnAxis(
                                ap=srcT_sb[:, col0:col0 + gb], axis=0),
                        )
                        for g in range(gb):
                            s = c0 + g
                            col = t * S + s
                            xl_g = gbuf[:, g * HC:(g + 1) * HC]
                            sel = wp.tile([P, P], dt.float32, tag="sel")
                            nc.vector.tensor_tensor(
                                out=sel[:],
                                in0=dstloc_sb[:, col:col + 1].to_broadcast([P, P]),
                                in1=iota_sb[:], op=Alu.is_equal)
                            selT_ps = pp1.tile([P, P], dt.float32, tag="small")
                            nc.tensor.transpose(out=selT_ps[:], in_=sel[:],
                                                identity=ident_sb[:])
                            selT = wp.tile([P, P], dt.float32, tag="selT")
                            nc.scalar.activation(out=selT[:], in_=selT_ps[:],
                                                 func=Act.Copy)
                            if tap_t and s == 0:
                                nc.sync.dma_start(out=dbg_sel[:, :], in_=sel[:])
                                nc.sync.dma_start(out=dbg_selT[:, :], in_=selT[:])
                            m_ps = pp.tile([P, HC], dt.float32, tag="big")
                            for n0, n1 in _chunks(HC):
                                nc.tensor.matmul(out=m_ps[:, n0:n1], lhsT=selT[:],
                                                 rhs=xr_sb[:, n0:n1],
                                                 start=True, stop=False)
                                nc.tensor.matmul(out=m_ps[:, n0:n1], lhsT=ident_sb[:],
                                                 rhs=xl_g[:, n0:n1],
                                                 start=False, stop=True)
                            # lrelu(m) = max(m, 0.2*m); the ISA allows only one
                            # PSUM operand per DVE op, so scale on ACT first.
                            msc = wp.tile([P, HC], dt.float32, tag="msc")
                            nc.scalar.activation(out=msc[:], in_=m_ps[:],
                                                 func=Act.Copy, scale=NS_ATT)
                            q = wp.tile([P, HC], dt.float32, tag="q")
                            nc.vector.tensor_tensor(out=q[:], in0=m_ps[:],
                                                    in1=msc[:], op=Alu.max)
                            tq = wp.tile([P, HC], dt.float32, tag="tq")
                            nc.vector.tensor_tensor(out=tq[:], in0=q[:], in1=att_sb[:],
                                                    op=Alu.mult)
                            s8 = wp.tile([P, H], dt.float32, tag="s8")
                            nc.vector.tensor_reduce(
                                out=s8[:], in_=tq[:].rearrange("p (h c) -> p h c", h=H),
                                axis=mybir.AxisListType.X, op=Alu.add)
                            ex = wp.tile([P, H], dt.float32, tag="ex")
                            nc.scalar.activation(out=ex[:], in_=s8[:], func=Act.Exp)
                            if tap_t and s == 0:
                                nc.sync.dma_start(out=dbg_lr[:, :], in_=q[:])
                                nc.sync.dma_start(out=dbg_s8[:, :], in_=s8[:])
                                nc.sync.dma_start(out=dbg_ex[:, :], in_=ex[:])
                            msg = wp.tile([P, HC], dt.float32, tag="msg")
                            nc.vector.tensor_tensor(
                                out=msg[:].rearrange("p (h c) -> p h c", h=H),
                                in0=xl_g.rearrange("p (h c) -> p h c", h=H),
                                in1=ex[:].to_broadcast([P, H, C]), op=Alu.mult)
                            if tap_t and s == 0:
                                nc.sync.dma_start(out=dbg_msg[:, :], in_=msg[:])
                            first, last = (s == 0), (s == S - 1)
                            for n0, n1 in _chunks(HC):
                                nc.tensor.matmul(out=out_ps[:, n0:n1], lhsT=sel[:],
                                                 rhs=msg[:, n0:n1],
                                                 start=first, stop=last)
                            nc.tensor.matmul(out=den_ps[:], lhsT=sel[:], rhs=ex[:],
                                             start=first, stop=last)

                    if tap_t:
                        den_sb = wp.tile([P, H], dt.float32, tag="densb")
                        nc.vector.tensor_copy(out=den_sb[:], in_=den_ps[:])
                        nc.sync.dma_start(out=dbg_den[:, :], in_=den_sb[:])
                    rden = wp.tile([P, H], dt.float32, tag="rden")
                    nc.vector.reciprocal(out=rden[:], in_=den_ps[:])
                    o_sb = wp.tile([P, HC], dt.float32, tag="o")
                    nc.vector.tensor_tensor(
                        out=o_sb[:].rearrange("p (h c) -> p h c", h=H),
                        in0=out_ps[:].rearrange("p (h c) -> p h c", h=H),
                        in1=rden[:].to_broadcast([P, H, C]), op=Alu.mult)
                    if tap_t:
                        nc.sync.dma_start(out=dbg_osb[:, :], in_=o_sb[:])
                    zsum = wp.tile([P, C], dt.float32, tag="zsum")
                    nc.vector.tensor_reduce(
                        out=zsum[:], in_=o_sb[:].rearrange("p (h c) -> p c h", h=H),
                        axis=mybir.AxisListType.X, op=Alu.add)
                    zt = wp.tile([P, C], dt.float32, tag="zt")
                    nc.vector.scalar_tensor_tensor(
                        out=zt[:], in0=zsum[:], scalar=inv_h, in1=bias_sb[:],
                        op0=Alu.mult, op1=Alu.add)
                    if act:
                        zt2 = wp.tile([P, C], dt.float32, tag="zt2")
                        nc.vector.scalar_tensor_tensor(
                            out=zt2[:], in0=zt[:], scalar=NS_ACT, in1=zt[:],
                            op0=Alu.mult, op1=Alu.max)
                    else:
                        zt2 = zt
                    ztp = pp1.tile([P, C], dt.float32, tag="small")
                    nc.tensor.transpose(out=ztp[:, :], in_=zt2[:], identity=ident_sb[:])
                    ztsb = wp.tile([C, P], dt.float32, tag="ztsb")
                    nc.scalar.activation(out=ztsb[:], in_=ztp[:, :], func=Act.Copy)
                    nc.sync.dma_start(out=zT_out[:, t * P:(t + 1) * P], in_=ztsb[:])

            # ------------------ pipeline ------------------
            PH = cfg.get("_phases", "ABCDEFGH")
            for _ in range(PH.count("A")):
                dense_phase(wl1_sb, blr1_sb, xl1_tab,
                            lambda nt: x_T[:, nt * P:(nt + 1) * P])
            for _ in range(PH.count("B")):
                edge_phase(xl1_tab, wr1_sb, brr1_sb, att1_sb, bias1_sb, zT_loc, act=True)
            for _ in range(PH.count("C")):
                nc.gpsimd.collective_compute(
                    "AllGather", Alu.bypass,
                    replica_groups=[list(range(NC_))],
                    ins=[zT_loc.opt()], outs=[zT_all.opt()])
            for _ in range(PH.count("D")):
                dense_phase(wl2_sb, blr2_sb, xl2_tab,
                            lambda nt: zT_all[nt // T, :, (nt % T) * P:(nt % T + 1) * P])
            for _ in range(PH.count("E")):
                edge_phase(xl2_tab, wr2_sb, brr2_sb, att2_sb, bias2_sb, z2T_loc, act=False)
            for _ in range(PH.count("F")):
                nc.gpsimd.collective_compute(
                    "AllGather", Alu.bypass,
                    replica_groups=[list(range(NC_))],
                    ins=[z2T_loc.opt()], outs=[z2T_all.opt()])

            if dbg:
                nc.gpsimd.dma_start(out=dbg_xl1[:, :], in_=xl1_tab[0:2 * P, :])
                nc.gpsimd.dma_start(out=dbg_zT[:, :], in_=zT_loc[:, :])
                nc.gpsimd.dma_start(
                    out=dbg_zall[:, :],
                    in_=zT_all[:, :, :].rearrange("k c n -> (k c) n"))
                nc.gpsimd.dma_start(out=dbg_xl2[:, :], in_=xl2_tab[0:2 * P, :])
                nc.gpsimd.dma_start(out=dbg_z2T[:, :], in_=z2T_loc[:, :])

            # ---- phase G: P/Q tables -------------------------------------------
            for nt in list(range(TN)) * PH.count("G"):
                lt = wp.tile([C, P], dt.float32, tag="lhsT")
                nc.sync.dma_start(
                    out=lt[:],
                    in_=z2T_all[nt // T, :, (nt % T) * P:(nt % T + 1) * P])
                psp = pp1.tile([P, C], dt.float32, tag="small")
                nc.tensor.matmul(out=psp[:], lhsT=ones_sb[:, :], rhs=bd1_sb[:, :],
                                 start=True, stop=False)
                nc.tensor.matmul(out=psp[:], lhsT=lt[:], rhs=a_sb[:],
                                 start=False, stop=True)
                p_sb = wp.tile([P, C], dt.float32, tag="pq_out")
                nc.scalar.activation(out=p_sb[:], in_=psp[:], func=Act.Copy)
                nc.sync.dma_start(out=p_tab[nt * P:(nt + 1) * P, :], in_=p_sb[:])
                psq = pp1.tile([P, C], dt.float32, tag="small")
                nc.tensor.matmul(out=psq[:], lhsT=lt[:], rhs=b_sb[:],
                                 start=True, stop=True)
                q_sb = wp.tile([P, C], dt.float32, tag="pq_out")
                nc.scalar.activation(out=q_sb[:], in_=psq[:], func=Act.Copy)
                nc.sync.dma_start(out=q_tab[nt * P:(nt + 1) * P, :], in_=q_sb[:])

            if dbg:
                nc.gpsimd.dma_start(out=dbg_p[:, :], in_=p_tab[0:2 * P, :])
                nc.gpsimd.dma_start(out=dbg_q[:, :], in_=q_tab[0:2 * P, :])

            # ---- phase H: decoder ----------------------------------------------
            bd2 = float(cfg["bd2"])
            for j0 in list(range(0, DSUB, GBD)) * PH.count("H"):
                gbd = min(GBD, DSUB - j0)
                pg = gp.tile([P, GBD * C], dt.float32, tag="pg")
                nc.gpsimd.indirect_dma_start(
                    out=pg[:, : gbd * C], out_offset=None, in_=p_tab[:, :],
                    in_offset=bass.IndirectOffsetOnAxis(
                        ap=dsrc_sb[:, j0:j0 + gbd], axis=0))
                qg = gp.tile([P, GBD * C], dt.float32, tag="qg")
                nc.gpsimd.indirect_dma_start(
                    out=qg[:, : gbd * C], out_offset=None, in_=q_tab[:, :],
                    in_offset=bass.IndirectOffsetOnAxis(
                        ap=ddst_sb[:, j0:j0 + gbd], axis=0))
                for g in range(gbd):
                    j = j0 + g
                    u = wp.tile([P, C], dt.float32, tag="u")
                    nc.vector.tensor_tensor(out=u[:], in0=pg[:, g * C:(g + 1) * C],
                                            in1=qg[:, g * C:(g + 1) * C], op=Alu.add)
                    d = wp.tile([P, C], dt.float32, tag="d")
                    nc.vector.scalar_tensor_tensor(
                        out=d[:], in0=u[:], scalar=NS_ACT, in1=u[:],
                        op0=Alu.mult, op1=Alu.max)
                    dw = wp.tile([P, C], dt.float32, tag="dw")
                    nc.vector.tensor_tensor(out=dw[:], in0=d[:], in1=wd2_sb[:],
                                            op=Alu.mult)
                    nc.vector.tensor_reduce(out=res_sb[:, j:j + 1], in_=dw[:],
                                            axis=mybir.AxisListType.X, op=Alu.add)

            # write result: transpose res_sb [P, DSUB] into out_dec [DSUB, P]
            for b0 in (range(0, DSUB, P) if "H" in PH else []):  # write once
                bw = min(P, DSUB - b0)
                rp = pp1.tile([P, P], dt.float32, tag="small")
                nc.tensor.transpose(out=rp[:bw, :], in_=res_sb[:, b0:b0 + bw],
                                    identity=ident_sb[:])
                rsb = wp.tile([P, P], dt.float32, tag="res_out")
                nc.scalar.activation(out=rsb[:bw, :], in_=rp[:bw, :], func=Act.Copy)
                nc.sync.dma_start(out=out_dec[b0:b0 + bw, :], in_=rsb[:bw, :])

    if fix:
        _fix_waits(nc)
    return nc


def build_truncated(cfg, phases, fix=True):
    """Build with only a prefix of phases, for timing bisection.
    phases: string subset-prefix of "ABCDEFGH"."""
    cfg = dict(cfg)
    cfg["_phases"] = phases
    return build_program(cfg, fix=fix)


# ---------------------------------------------------------------------------
def host_prep(inputs, n_cores=8, GB=1, GBD=1):
    """Host-side preprocessing: edge sort/pad, weight transposes, per-core maps."""
    x = np.ascontiguousarray(np.asarray(inputs["x"], dtype=np.float32))
    N, C = x.shape
    Wl1 = np.asarray(inputs["Wl1"], np.float32)
    H = Wl1.shape[0] // C
    HC = H * C
    NPC = N // n_cores
    T = NPC // P

    ei = np.asarray(inputs["edge_index"])
    src = ei[0].astype(np.int64)
    dst = ei[1].astype(np.int64)
    E = src.shape[0]
    loops = np.arange(N, dtype=np.int64)
    src_a = np.concatenate([src, loops])
    dst_a = np.concatenate([dst, loops])
    order = np.argsort(dst_a, kind="stable")
    src_s, dst_s = src_a[order], dst_a[order]

    TN = N // P
    tile_id = dst_s // P
    counts = np.bincount(tile_id, minlength=TN)
    S_sub = int(np.ceil(counts.max() / P))
    cap = S_sub * P
    src_pad = np.zeros((TN, cap), np.int32)
    dstloc_pad = np.full((TN, cap), -1.0, np.float32)
    off = np.concatenate([[0], np.cumsum(counts)])
    for t in range(TN):
        c = counts[t]
        src_pad[t, :c] = src_s[off[t]:off[t] + c]
        dstloc_pad[t, :c] = (dst_s[off[t]:off[t] + c] - t * P).astype(np.float32)

    E_dec = E // n_cores
    assert E % n_cores == 0 and E_dec % P == 0
    DSUB = E_dec // P

    def tr(a):  # -> f32 transposed contiguous
        return np.ascontiguousarray(np.asarray(a, np.float32).T)

    Wd1 = np.asarray(inputs["Wd1"], np.float32)
    shared = {
        "x_T": tr(x),
        "wl1T": tr(inputs["Wl1"]), "wr1T": tr(inputs["Wr1"]),
        "wl2T": tr(inputs["Wl2"]), "wr2T": tr(inputs["Wr2"]),
        "blr1": np.asarray(inputs["bl1"], np.float32).reshape(1, HC),
        "brr1": np.asarray(inputs["br1"], np.float32).reshape(1, HC),
        "blr2": np.asarray(inputs["bl2"], np.float32).reshape(1, HC),
        "brr2": np.asarray(inputs["br2"], np.float32).reshape(1, HC),
        "att1_t": np.tile(np.asarray(inputs["att1"], np.float32).reshape(1, HC), (P, 1)),
        "att2_t": np.tile(np.asarray(inputs["att2"], np.float32).reshape(1, HC), (P, 1)),
        "bias1_t": np.tile(np.asarray(inputs["bias1"], np.float32).reshape(1, C), (P, 1)),
        "bias2_t": np.tile(np.asarray(inputs["bias2"], np.float32).reshape(1, C), (P, 1)),
        "a_t": np.ascontiguousarray(Wd1[:, :C].T),
        "b_t": np.ascontiguousarray(Wd1[:, C:].T),
        "bd1r": np.asarray(inputs["bd1"], np.float32).reshape(1, C),
        "wd2_t": np.tile(np.asarray(inputs["Wd2"], np.float32).reshape(1, C), (P, 1)),
        "iota_t": np.tile(np.arange(P, dtype=np.float32)[None, :], (P, 1)),
        "ident_t": np.eye(P, dtype=np.float32),
        "ones_t": np.ones((1, P), np.float32),
    }

    xt_full = shared["x_T"]
    in_maps = []
    for k in range(n_cores):
        tiles = slice(k * T, (k + 1) * T)
        src_k = np.ascontiguousarray(
            src_pad[tiles].reshape(T * S_sub, P).T)           # [P, T*S]
        dl_k = np.ascontiguousarray(
            dstloc_pad[tiles].reshape(T * S_sub, P).T)
        es = slice(k * E_dec, (k + 1) * E_dec)
        dsrc_k = np.ascontiguousarray(
            src[es].astype(np.int32).reshape(DSUB, P).T)
        ddst_k = np.ascontiguousarray(
            dst[es].astype(np.int32).reshape(DSUB, P).T)
        m = dict(shared)
        m["x_own_T"] = np.ascontiguousarray(xt_full[:, k * NPC:(k + 1) * NPC])
        m["srcT"] = src_k
        m["dstlocT"] = dl_k
        m["dec_srcT"] = dsrc_k
        m["dec_dstT"] = ddst_k
        in_maps.append(m)

    cfg = {
        "N": N, "C": C, "H": H, "n_cores": n_cores, "S_sub": S_sub,
        "DSUB": DSUB, "GB": GB, "GBD": GBD,
        "bd2": float(np.asarray(inputs["bd2"]).reshape(-1)[0]),
    }
    return in_maps, cfg


def postprocess(results, cfg):
    """results: list (per core) of dicts with 'out_dec' -> full output."""
    n_cores = cfg["n_cores"]
    out = np.concatenate(
        [results[k]["out_dec"].reshape(-1) for k in range(n_cores)])
    return (out + cfg["bd2"]).astype(np.float32)


def kernel(**inputs):
    from concourse.bass_utils import run_bass_kernel_spmd

    n_cores = 8
    in_maps, cfg = host_prep(inputs, n_cores=n_cores)
    nc = build_program(cfg)
    res = run_bass_kernel_spmd(nc, in_maps, list(range(n_cores)))
    return postprocess(res.results, cfg)



# revision 23
# speedup vs baseline: 1.0873x; 1.0873x over previous
"""GATv2 x2 + edge decoder (gnn_message_passing) on 8 TRN2 NeuronCores.

v4 design (bf16, gather-accumulate, dst-sharded edges):
- Per layer, dense phase computes TWO node tables in DRAM (bf16):
  xl_tab = x @ Wl.T (no bias; alpha sums to 1 so bl folds into the output
  bias) and xr_tab = x @ Wr.T (+ bl+br folded in, covering the m-bias).
- Edge phase (dst-sorted edges, 128-edge subtiles, core k owns dst nodes
  [k*N/8,(k+1)*N/8)): per subtile one indirect DMA gathers xl[src] rows and
  a second indirect DMA with compute_op=add accumulates xr[dst] in-flight,
  producing m = xl[src]+xr[dst] directly in SBUF with zero compute ops.
- Scores: q = Lrelu(m) on ACT, tq = q*att and the per-head reduce on DVE,
  batched over G subtiles per instruction. exp on ACT.
- Scatter (segment softmax sum): one matmul per subtile with the 0/1 matrix
  sel[e,n] = (dstloc_e == n) as the stationary operand accumulates both
  out += sel.T @ (xl[src]*ex) and den += sel.T @ ex in PSUM; the divide by
  den happens once per 128-node dst tile.
- Host-side node re-permutation equalizes per-dst-tile edge counts
  (S_sub = 18 instead of 19).
- z is AllGathered between layers (bf16); layer-2/PQ lhsT tiles are loaded
  with DMA-transpose.
- Decoder: P/Q node tables (256B bf16 rows); per 128-edge subtile one
  gather P[src] + one CCE-add gather Q[dst], then lrelu/dot/reduce on DVE
  batched over 4 subtiles.
"""

import sys

sys.path.insert(0, "/opt/trn_rl_repo")

import numpy as np

import bass_rust
import concourse.bass as bass
import concourse.mybir as mybir
import concourse.tile as tile

P = 128
NS_ATT = 0.2
NS_ACT = 0.01
dt = mybir.dt
Alu = mybir.AluOpType
Act = mybir.ActivationFunctionType


# ---------------------------------------------------------------------------
# workaround: this walrus build rejects sem waits attached to InstDrain
# ("Too many sync wait commands"); hoist every drain wait onto NoOps.
def _fix_waits(nc, max_other=1):
    for bb in nc.main_func.blocks:
        newlist = []
        for ins in bb.instructions:
            si = ins.sync_info
            if si is not None and si.on_wait:
                waits = list(si.on_wait)
                no_wait = isinstance(ins, mybir.InstDrain) or hasattr(ins, "isa_opcode")
                limit = 0 if no_wait else max_other
                if len(waits) > limit:
                    nkeep = limit
                    extra = waits[: len(waits) - nkeep] if nkeep else waits
                    keep = waits[len(waits) - nkeep:] if nkeep else []
                    k = 0
                    while extra:
                        chunk, extra = extra[:1], extra[1:]
                        nop = mybir.InstNoOp(
                            name=f"{ins.name}_ws{k}", engine=ins.engine, ins=[], outs=[]
                        )
                        nop.sync_info = bass_rust.SyncInfo(on_wait=chunk, on_update=[])
                        newlist.append(nop)
                        k += 1
                    ins.sync_info = bass_rust.SyncInfo(
                        on_wait=keep, on_update=list(si.on_update or [])
                    )
            newlist.append(ins)
        bb.instructions = newlist


def _batches(S, G):
    """Split S subtiles into batches of size <= G, e.g. 19,6 -> [7,6,6]."""
    nb = (S + G - 1) // G
    base, rem = divmod(S, nb)
    return [base + (1 if i < rem else 0) for i in range(nb)]


def build_program(cfg, fix=True, dbg=False):
    N, C, H = cfg["N"], cfg["C"], cfg["H"]
    NC_ = cfg["n_cores"]
    HC = H * C
    NPC = N // NC_
    T = NPC // P
    TN = N // P
    S = cfg["S_sub"]
    G = cfg["G"]
    GD = cfg["GD"]
    DSUB = cfg["DSUB"]
    BD = 2                      # dense-phase tile batch
    has_mbias1 = cfg["has_mbias1"]
    has_mbias2 = cfg["has_mbias2"]
    has_bd1 = cfg["has_bd1"]
    bat = _batches(S, G)
    Gmax = bat[0]

    nc = bass.Bass()
    bf = dt.bfloat16
    f32 = dt.float32

    def inp(name, shape, d=bf):
        return nc.declare_dram_parameter(name, list(shape), d, isOutput=False)

    xT = inp("xT", [C, N])
    wlT1 = inp("wlT1", [C, HC])
    wrT1 = inp("wrT1", [C, HC])
    wlT2 = inp("wlT2", [C, HC])
    wrT2 = inp("wrT2", [C, HC])
    att1r = inp("att1r", [P, Gmax * HC])
    att2r = inp("att2r", [P, Gmax * HC])
    iotar = inp("iotar", [P, Gmax * P])
    dstloc = inp("dstloc", [P, T * S])
    srcI = inp("srcI", [P, T * S], dt.int32)
    dstI = inp("dstI", [P, T * S], dt.int32)
    bias1r = inp("bias1r", [P, C], f32)
    bias2r = inp("bias2r", [P, C], f32)
    blr1 = inp("blr1", [1, HC], f32)     # bl1+br1 (m-bias layer 1)
    blr2 = inp("blr2", [1, HC], f32)
    onesr = inp("onesr", [1, P], f32)
    abT = inp("abT", [C, 2 * C])
    bd1r = inp("bd1r", [1, 2 * C], f32)
    wd2r = inp("wd2r", [P, GD * C])
    decS = inp("decS", [P, DSUB], dt.int32)
    decD = inp("decD", [P, DSUB], dt.int32)
    selfI = inp("selfI", [P, T], dt.int32)
    iotac = inp("iotac", [P, 1], f32)

    out_dec = nc.declare_dram_parameter("out_dec", [DSUB, P], f32, isOutput=True)
    if dbg:
        d_xl1 = nc.declare_dram_parameter("d_xl1", [2 * P, HC], f32, isOutput=True)
        d_xr1 = nc.declare_dram_parameter("d_xr1", [2 * P, HC], f32, isOutput=True)
        d_gl = nc.declare_dram_parameter("d_gl", [P, 6 * HC], f32, isOutput=True)
        d_sel = nc.declare_dram_parameter("d_sel", [P, 6 * P], f32, isOutput=True)
        d_q = nc.declare_dram_parameter("d_q", [P, 6 * HC], f32, isOutput=True)
        d_s8 = nc.declare_dram_parameter("d_s8", [P, 6 * H], f32, isOutput=True)
        d_ex = nc.declare_dram_parameter("d_ex", [P, 6 * H], f32, isOutput=True)
        d_msg = nc.declare_dram_parameter("d_msg", [P, 6 * HC], f32, isOutput=True)
        d_den = nc.declare_dram_parameter("d_den", [P, H], f32, isOutput=True)
        d_z = nc.declare_dram_parameter("d_z", [NPC, C], f32, isOutput=True)
        d_zall = nc.declare_dram_parameter("d_zall", [2 * P, C], f32, isOutput=True)
        d_p = nc.declare_dram_parameter("d_p", [2 * P, C], f32, isOutput=True)
        d_res = nc.declare_dram_parameter("d_res", [P, DSUB], f32, isOutput=True)
        d_o = nc.declare_dram_parameter("d_o", [P, HC], f32, isOutput=True)
        d_zsum = nc.declare_dram_parameter("d_zsum", [P, C], f32, isOutput=True)
        d_rden = nc.declare_dram_parameter("d_rden", [P, H], f32, isOutput=True)
        d_outps = nc.declare_dram_parameter("d_outps", [P, HC], f32, isOutput=True)
        d_outp1 = nc.declare_dram_parameter("d_outp1", [P, HC], f32, isOutput=True)

    with tile.TileContext(nc) as tc:
        with (
            tc.tile_pool(name="const", bufs=1) as cp,
            tc.tile_pool(name="work", bufs=2) as wp,
            tc.tile_pool(name="msgp", bufs=3) as mp,
            tc.tile_pool(name="gpool", bufs=3) as gp,
            tc.tile_pool(name="psum", bufs=3, space="PSUM") as pp,
            tc.tile_pool(name="psum1", bufs=2, space="PSUM") as pp1,
            tc.tile_pool(name="dram", bufs=1, space="DRAM") as dp,
        ):
            def cload(ap, shape, d=bf):
                t_ = cp.tile(list(shape), d, tag=f"c_{ap.name}")
                nc.sync.dma_start(out=t_[:], in_=ap[:, :])
                return t_

            wl1_sb = cload(wlT1, [C, HC])
            wr1_sb = cload(wrT1, [C, HC])
            wl2_sb = cload(wlT2, [C, HC])
            wr2_sb = cload(wrT2, [C, HC])
            def load_att(ap):
                t_ = cp.tile([P, Gmax * HC], bf, tag="c_att")
                nc.sync.dma_start(out=t_[:], in_=ap[:, :])
                return t_

            att1_sb = load_att(att1r)
            iota_sb = cload(iotar, [P, Gmax * P])
            dstloc_sb = cload(dstloc, [P, T * S])
            srcI_sb = cload(srcI, [P, T * S], dt.int32)
            dstI_sb = cload(dstI, [P, T * S], dt.int32)
            bias1_sb = cload(bias1r, [P, C], f32)
            bias2_sb = cload(bias2r, [P, C], f32)
            blr1_sb = cload(blr1, [1, HC], f32)
            blr2_sb = cload(blr2, [1, HC], f32)
            ones_sb = cload(onesr, [1, P], f32)
            abT_sb = cload(abT, [C, 2 * C])
            bd1_sb = cload(bd1r, [1, 2 * C], f32)
            wd2_sb = cload(wd2r, [P, GD * C])
            decS_sb = cload(decS, [P, DSUB], dt.int32)
            selfI_sb = cload(selfI, [P, T], dt.int32)
            decD_sb = cload(decD, [P, DSUB], dt.int32)
            res_sb = cp.tile([P, DSUB], f32, tag="res_sb")

            xl1_tab = dp.tile([N, HC], bf)
            xr1_tab = dp.tile([N, HC], bf)
            xl2_tab = dp.tile([N, HC], bf)
            xr2_tab = dp.tile([N, HC], bf)
            z_loc = dp.tile([NPC, C], bf)
            z_all = dp.tile([NC_ * NPC, C], bf)
            z2_loc = dp.tile([NPC, C], bf)
            z2_all = dp.tile([NC_ * NPC, C], bf)
            p_tab = dp.tile([N, C], bf)
            q_tab = dp.tile([N, C], bf)

            # ---------------- dense phase: xl/xr tables --------------------
            def dense_phase(wlT_sb, wrT_sb, mbias_sb, has_mbias, xl_tab, xr_tab,
                            load_lhsT):
                for b0 in range(0, TN, BD):
                    lt = wp.tile([C, BD * P], bf, tag="lhsT")
                    load_lhsT(lt, b0)
                    xsl = wp.tile([P, BD * HC], bf, tag="xsl")
                    xsr = wp.tile([P, BD * HC], bf, tag="xsr")
                    for i in range(BD):
                        psl = pp.tile([P, HC], f32, tag="big")
                        for n0 in range(0, HC, 512):
                            nc.tensor.matmul(out=psl[:, n0:n0 + 512],
                                             lhsT=lt[:, i * P:(i + 1) * P],
                                             rhs=wlT_sb[:, n0:n0 + 512],
                                             start=True, stop=True)
                        psr = pp.tile([P, HC], f32, tag="big")
                        if has_mbias:
                            for n0 in range(0, HC, 512):
                                nc.tensor.matmul(out=psr[:, n0:n0 + 512],
                                                 lhsT=ones_sb[:, :],
                                                 rhs=mbias_sb[:, n0:n0 + 512],
                                                 start=True, stop=False)
                            for n0 in range(0, HC, 512):
                                nc.tensor.matmul(out=psr[:, n0:n0 + 512],
                                                 lhsT=lt[:, i * P:(i + 1) * P],
                                                 rhs=wrT_sb[:, n0:n0 + 512],
                                                 start=False, stop=True)
                        else:
                            for n0 in range(0, HC, 512):
                                nc.tensor.matmul(out=psr[:, n0:n0 + 512],
                                                 lhsT=lt[:, i * P:(i + 1) * P],
                                                 rhs=wrT_sb[:, n0:n0 + 512],
                                                 start=True, stop=True)
                        nc.scalar.activation(out=xsl[:, i * HC:(i + 1) * HC],
                                             in_=psl[:], func=Act.Copy)
                        nc.scalar.activation(out=xsr[:, i * HC:(i + 1) * HC],
                                             in_=psr[:], func=Act.Copy)
                    rows = slice(b0 * P, (b0 + BD) * P)
                    nc.sync.dma_start(
                        out=xl_tab[rows, :].rearrange("(b p) c -> p b c", p=P),
                        in_=xsl[:].rearrange("p (b c) -> p b c", b=BD))
                    nc.sync.dma_start(
                        out=xr_tab[rows, :].rearrange("(b p) c -> p b c", p=P),
                        in_=xsr[:].rearrange("p (b c) -> p b c", b=BD))

            # ---------------- edge phase ----------------------------------
            def edge_phase(xl_tab, xr_tab, att_sb, bias_sb, z_out, act):
                layer1 = z_out is z_loc
                inv_h = 1.0 / H
                for t in range(T):
                    tap = dbg and layer1 and t == 0
                    out_ps = pp.tile([P, HC], f32, tag="big")
                    den_ps = pp1.tile([P, 2 * C], f32, tag="small")
                    s = 0
                    for g in bat:
                        col0 = t * S + s
                        gl = gp.tile([P, Gmax, HC], bf, tag="gl")
                        for j in range(g):
                            nc.gpsimd.indirect_dma_start(
                                out=gl[:, j, :], out_offset=None, in_=xl_tab[:, :],
                                in_offset=bass.IndirectOffsetOnAxis(
                                    ap=srcI_sb[:, col0 + j:col0 + j + 1], axis=0))
                            nc.gpsimd.indirect_dma_start(
                                out=gl[:, j, :], out_offset=None, in_=xr_tab[:, :],
                                in_offset=bass.IndirectOffsetOnAxis(
                                    ap=dstI_sb[:, col0 + j:col0 + j + 1], axis=0),
                                compute_op=Alu.add)
                        sel = wp.tile([P, Gmax * P], bf, tag="sel")
                        nc.vector.tensor_tensor(
                            out=sel[:, :g * P].rearrange("p (g n) -> p g n", g=g),
                            in0=dstloc_sb[:, col0:col0 + g].to_broadcast([P, g, P]),
                            in1=iota_sb[:, :g * P].rearrange("p (g n) -> p g n", g=g),
                            op=Alu.is_equal)
                        q = wp.tile([P, Gmax * HC], bf, tag="q")
                        nc.scalar.activation(
                            out=q[:, :g * HC],
                            in_=gl[:, :, :].rearrange("p g c -> p (g c)")[:, :g * HC],
                            func=Act.Prelu, alpha=NS_ATT)
                        tq = wp.tile([P, Gmax * HC], bf, tag="tq")
                        nc.vector.tensor_tensor(out=tq[:, :g * HC], in0=q[:, :g * HC],
                                                in1=att_sb[:, :g * HC], op=Alu.mult)
                        s8 = wp.tile([P, Gmax * H], f32, tag="s8")
                        nc.vector.tensor_reduce(
                            out=s8[:, :g * H],
                            in_=tq[:, :g * HC].rearrange(
                                "p (gh c) -> p gh c", c=C),
                            axis=mybir.AxisListType.X, op=Alu.add)
                        ex = wp.tile([P, Gmax * H], bf, tag="ex")
                        nc.scalar.activation(out=ex[:, :g * H], in_=s8[:, :g * H],
                                             func=Act.Exp)
                        msg = mp.tile([P, Gmax * HC], bf, tag="msg")
                        for j in range(g):
                            nc.vector.tensor_tensor(
                                out=msg[:, j * HC:(j + 1) * HC]
                                    .rearrange("p (h c) -> p h c", h=H),
                                in0=gl[:, j, :].rearrange("p (h c) -> p h c", h=H),
                                in1=ex[:, j * H:(j + 1) * H].to_broadcast([P, H, C]),
                                op=Alu.mult)
                        for j in range(g):
                            first, last = (s + j == 0), (s + j == S - 1)
                            if tap and s + j == 1:
                                o1 = wp.tile([P, HC], f32, tag="dbgo")
                                nc.vector.tensor_copy(out=o1[:], in_=out_ps[:])
                                nc.sync.dma_start(out=d_outp1[:, :], in_=o1[:, :HC])
                            for n0 in range(0, HC, 512):
                                nc.tensor.matmul(
                                    out=out_ps[:, n0:n0 + 512],
                                    lhsT=sel[:, j * P:(j + 1) * P],
                                    rhs=msg[:, j * HC + n0:j * HC + n0 + 512],
                                    start=first, stop=last)
                            nc.tensor.matmul(
                                out=den_ps[:, :H],
                                lhsT=sel[:, j * P:(j + 1) * P],
                                rhs=ex[:, j * H:(j + 1) * H],
                                start=first, stop=last)
                        if tap and s == 0:
                            def dmp(dst, src_bf, width):
                                w2 = min(width, HC)
                                tmp = wp.tile([P, HC], f32, tag="dbgo")
                                nc.vector.tensor_copy(out=tmp[:, :w2], in_=src_bf[:, :w2])
                                nc.sync.dma_start(out=dst[:, 0:w2], in_=tmp[:, :w2])
                            dmp(d_gl, msg[:, 2 * HC:], 4 * HC)
                            dmp(d_sel, sel[:, :g * P], g * P)
                            dmp(d_q, msg[:, 4 * HC:], 2 * HC)
                            dmp(d_s8, s8[:, :g * H], g * H)
                            dmp(d_ex, ex[:, :g * H], g * H)
                            dmp(d_msg, msg[:, :g * HC], g * HC)
                        s += g
                    xrt = wp.tile([P, HC], bf, tag="xrt")
                    nc.gpsimd.indirect_dma_start(
                        out=xrt[:], out_offset=None, in_=xr_tab[:, :],
                        in_offset=bass.IndirectOffsetOnAxis(
                            ap=selfI_sb[:, t:t + 1], axis=0))
                    rden = wp.tile([P, H], f32, tag="rden")
                    if tap:
                        ot = wp.tile([P, HC], f32, tag="dbgo")
                        nc.vector.tensor_copy(out=ot[:], in_=out_ps[:])
                        nc.sync.dma_start(out=d_outps[:, :], in_=ot[:])
                        dent = wp.tile([P, H], f32, tag="dent")
                        nc.vector.tensor_copy(out=dent[:], in_=den_ps[:, :H])
                        nc.sync.dma_start(out=d_den[:, :], in_=dent[:])
                    nc.vector.reciprocal(out=rden[:], in_=den_ps[:, :H])
                    o_sb = wp.tile([P, HC], bf, tag="o")
                    nc.vector.tensor_tensor(
                        out=o_sb[:].rearrange("p (h c) -> p h c", h=H),
                        in0=out_ps[:].rearrange("p (h c) -> p h c", h=H),
                        in1=rden[:].to_broadcast([P, H, C]), op=Alu.mult)
                    if tap:
                        nc.sync.dma_start(out=d_rden[:, :], in_=rden[:])
                        ot2 = wp.tile([P, HC], f32, tag="dbgo")
                        nc.vector.tensor_copy(out=ot2[:], in_=o_sb[:])
                        nc.sync.dma_start(out=d_o[:, :], in_=ot2[:])
                    o2 = wp.tile([P, HC], bf, tag="o2")
                    nc.vector.tensor_tensor(out=o2[:], in0=o_sb[:], in1=xrt[:],
                                            op=Alu.subtract)
                    zsum = wp.tile([P, C], f32, tag="zsum")
                    nc.vector.tensor_reduce(
                        out=zsum[:], in_=o2[:].rearrange("p (h c) -> p c h", h=H),
                        axis=mybir.AxisListType.X, op=Alu.add)
                    if tap:
                        nc.sync.dma_start(out=d_zsum[:, :], in_=zsum[:])
                    zt = wp.tile([P, C], f32 if act else bf, tag="zt")
                    nc.vector.scalar_tensor_tensor(
                        out=zt[:], in0=zsum[:], scalar=inv_h, in1=bias_sb[:],
                        op0=Alu.mult, op1=Alu.add)
                    if act:
                        zf = wp.tile([P, C], bf, tag="zf")
                        nc.vector.scalar_tensor_tensor(
                            out=zf[:], in0=zt[:], scalar=NS_ACT, in1=zt[:],
                            op0=Alu.mult, op1=Alu.max)
                    else:
                        zf = zt
                    nc.sync.dma_start(out=z_out[t * P:(t + 1) * P, :], in_=zf[:])

            # ------------------- run -------------------
            dense_phase(wl1_sb, wr1_sb, blr1_sb, has_mbias1, xl1_tab, xr1_tab,
                        lambda lt, b0: nc.sync.dma_start(
                            out=lt[:], in_=xT[:, b0 * P:(b0 + BD) * P]))
            edge_phase(xl1_tab, xr1_tab, att1_sb, bias1_sb, z_loc, act=True)
            nc.gpsimd.collective_compute(
                "AllGather", Alu.bypass, replica_groups=[list(range(NC_))],
                ins=[z_loc.opt()], outs=[z_all.opt()])

            if dbg:
                nc.gpsimd.dma_start(out=d_xl1[:, :], in_=xl1_tab[0:2 * P, :])
                nc.gpsimd.dma_start(out=d_xr1[:, :], in_=xr1_tab[0:2 * P, :])
                nc.gpsimd.dma_start(out=d_z[:, :], in_=z_loc[:, :])
                nc.gpsimd.dma_start(out=d_zall[:, :], in_=z_all[0:2 * P, :])
            att2_sb = load_att(att2r)
            dense_phase(wl2_sb, wr2_sb, blr2_sb, has_mbias2, xl2_tab, xr2_tab,
                        lambda lt, b0: nc.sync.dma_start_transpose(
                            out=lt[:], in_=z_all[b0 * P:(b0 + BD) * P, :]))
            edge_phase(xl2_tab, xr2_tab, att2_sb, bias2_sb, z2_loc, act=False)
            nc.gpsimd.collective_compute(
                "AllGather", Alu.bypass, replica_groups=[list(range(NC_))],
                ins=[z2_loc.opt()], outs=[z2_all.opt()])

            # ------------------- P/Q tables -------------------
            for b0 in range(0, TN, BD):
                lt = wp.tile([C, BD * P], bf, tag="lhsT")
                nc.sync.dma_start_transpose(
                    out=lt[:], in_=z2_all[b0 * P:(b0 + BD) * P, :])
                pq = wp.tile([P, BD * 2 * C], bf, tag="pq")
                for i in range(BD):
                    ps = pp1.tile([P, 2 * C], f32, tag="small")
                    if has_bd1:
                        nc.tensor.matmul(out=ps[:], lhsT=ones_sb[:, :],
                                         rhs=bd1_sb[:, :], start=True, stop=False)
                        nc.tensor.matmul(out=ps[:], lhsT=lt[:, i * P:(i + 1) * P],
                                         rhs=abT_sb[:], start=False, stop=True)
                    else:
                        nc.tensor.matmul(out=ps[:], lhsT=lt[:, i * P:(i + 1) * P],
                                         rhs=abT_sb[:], start=True, stop=True)
                    nc.scalar.activation(out=pq[:, i * 2 * C:(i + 1) * 2 * C],
                                         in_=ps[:], func=Act.Copy)
                rows = slice(b0 * P, (b0 + BD) * P)
                nc.sync.dma_start(
                    out=p_tab[rows, :].rearrange("(b p) c -> p b c", p=P),
                    in_=pq[:].rearrange("p (b two c) -> p b two c", b=BD, two=2)
                        [:, :, 0, :])
                nc.sync.dma_start(
                    out=q_tab[rows, :].rearrange("(b p) c -> p b c", p=P),
                    in_=pq[:].rearrange("p (b two c) -> p b two c", b=BD, two=2)
                        [:, :, 1, :])

            # ------------------- decoder -------------------
            for j0 in range(0, DSUB, GD):
                u = gp.tile([P, GD, C], bf, tag="u")
                for j in range(GD):
                    nc.gpsimd.indirect_dma_start(
                        out=u[:, j, :], out_offset=None, in_=p_tab[:, :],
                        in_offset=bass.IndirectOffsetOnAxis(
                            ap=decS_sb[:, j0 + j:j0 + j + 1], axis=0))
                    nc.gpsimd.indirect_dma_start(
                        out=u[:, j, :], out_offset=None, in_=q_tab[:, :],
                        in_offset=bass.IndirectOffsetOnAxis(
                            ap=decD_sb[:, j0 + j:j0 + j + 1], axis=0),
                        compute_op=Alu.add)
                d = wp.tile([P, GD * C], bf, tag="d")
                uv = u[:, :, :].rearrange("p g c -> p (g c)")
                nc.vector.scalar_tensor_tensor(
                    out=d[:], in0=uv, scalar=NS_ACT, in1=uv,
                    op0=Alu.mult, op1=Alu.max)
                dw = wp.tile([P, GD * C], bf, tag="dw")
                nc.vector.tensor_tensor(out=dw[:], in0=d[:], in1=wd2_sb[:],
                                        op=Alu.mult)
                nc.vector.tensor_reduce(
                    out=res_sb[:, j0:j0 + GD],
                    in_=dw[:].rearrange("p (g c) -> p g c", g=GD),
                    axis=mybir.AxisListType.X, op=Alu.add)

            if dbg:
                nc.gpsimd.dma_start(out=d_p[:, :], in_=p_tab[0:2 * P, :])
                nc.sync.dma_start(out=d_res[:, :], in_=res_sb[:])

            # write result transposed: res_sb [P, DSUB] -> out_dec [DSUB, P]
            iotac_sb = cload(iotac, [P, 1], f32)
            ident_sb = cp.tile([P, P], f32, tag="ident")
            nc.vector.tensor_tensor(
                out=ident_sb[:],
                in0=iotac_sb[:, 0:1].to_broadcast([P, P]),
                in1=iota_sb[:, 0:P], op=Alu.is_equal)
            for b0 in range(0, DSUB, P):
                bw = min(P, DSUB - b0)
                rp = pp1.tile([P, 2 * C], f32, tag="small")
                nc.tensor.transpose(out=rp[:bw, :bw], in_=res_sb[:, b0:b0 + bw],
                                    identity=ident_sb[:])
                rsb = wp.tile([P, P], f32, tag="res_out")
                nc.scalar.activation(out=rsb[:bw, :], in_=rp[:bw, :P], func=Act.Copy)
                nc.sync.dma_start(out=out_dec[b0:b0 + bw, :], in_=rsb[:bw, :])

    if fix:
        _fix_waits(nc)
    return nc


# ---------------------------------------------------------------------------
def _rebalance_nodes(dst_counts, TN):
    """Greedy FFD: assign nodes to TN tiles of 128 nodes, equalizing edge
    counts. Returns perm (new_id -> old_id is inv; perm[old]=new)."""
    N = dst_counts.shape[0]
    order = np.argsort(-dst_counts, kind="stable")
    tile_load = np.zeros(TN, np.int64)
    tile_fill = np.zeros(TN, np.int32)
    # heap-free greedy: always put next node into least-loaded non-full tile
    import heapq
    heap = [(0, 0, t) for t in range(TN)]
    heapq.heapify(heap)
    perm = np.empty(N, np.int64)
    for node in order:
        while True:
            load, fill, t = heapq.heappop(heap)
            if tile_fill[t] < P:
                break
        perm[node] = t * P + tile_fill[t]
        tile_fill[t] += 1
        tile_load[t] += dst_counts[node]
        if tile_fill[t] < P:
            heapq.heappush(heap, (int(tile_load[t]), int(tile_fill[t]), t))
    return perm


def host_prep(inputs, n_cores=8, G=6, GD=4):
    x = np.asarray(inputs["x"], dtype=np.float32)
    N, C = x.shape
    Wl1 = np.asarray(inputs["Wl1"], np.float32)
    H = Wl1.shape[0] // C
    HC = H * C
    NPC = N // n_cores
    T = NPC // P
    TN = N // P

    ei = np.asarray(inputs["edge_index"])
    src0 = ei[0].astype(np.int64)
    dst0 = ei[1].astype(np.int64)
    E = src0.shape[0]
    loops = np.arange(N, dtype=np.int64)

    # --- node re-permutation for tile load balancing (GNN edges + loops) ---
    cnt = np.bincount(dst0, minlength=N) + 1
    perm = _rebalance_nodes(cnt, TN)          # perm[old_id] = new_id
    src = perm[src0]
    dst = perm[dst0]
    src_a = np.concatenate([src, perm[loops]])
    dst_a = np.concatenate([dst, perm[loops]])

    order = np.argsort(dst_a, kind="stable")
    src_s, dst_s = src_a[order], dst_a[order]

    tile_id = dst_s // P
    counts = np.bincount(tile_id, minlength=TN)
    S_sub = int(np.ceil(counts.max() / P))
    src_pad = np.zeros((TN, S_sub * P), np.int32)
    dstloc_pad = np.full((TN, S_sub * P), -1.0, np.float32)
    dstg_pad = np.zeros((TN, S_sub * P), np.int32)
    off = np.concatenate([[0], np.cumsum(counts)])
    for t in range(TN):
        c = counts[t]
        src_pad[t, :c] = src_s[off[t]:off[t] + c]
        dstloc_pad[t, :c] = (dst_s[off[t]:off[t] + c] - t * P).astype(np.float32)
        dstg_pad[t, :c] = dst_s[off[t]:off[t] + c]

    E_dec = E // n_cores
    assert E % n_cores == 0 and E_dec % P == 0
    DSUB = E_dec // P

    import ml_dtypes

    def bf(a):
        return np.asarray(a, np.float32).astype(ml_dtypes.bfloat16)

    def trbf(a):
        return bf(np.ascontiguousarray(np.asarray(a, np.float32).T))

    bl1 = np.asarray(inputs["bl1"], np.float32)
    br1 = np.asarray(inputs["br1"], np.float32)
    bl2 = np.asarray(inputs["bl2"], np.float32)
    br2 = np.asarray(inputs["br2"], np.float32)
    bias1 = np.asarray(inputs["bias1"], np.float32)
    bias2 = np.asarray(inputs["bias2"], np.float32)
    att1 = np.asarray(inputs["att1"], np.float32).reshape(1, HC)
    att2 = np.asarray(inputs["att2"], np.float32).reshape(1, HC)
    Wd1 = np.asarray(inputs["Wd1"], np.float32)
    bd1 = np.asarray(inputs["bd1"], np.float32)
    Wd2 = np.asarray(inputs["Wd2"], np.float32).reshape(C)

    bat = _batches(S_sub, G)
    Gmax = bat[0]
    x_perm = np.empty_like(x)
    x_perm[perm] = x                      # row new_id = x[old_id]

    b1eff = bias1 + bl1.reshape(H, C).mean(0)
    b2eff = bias2 + bl2.reshape(H, C).mean(0)
    mb1 = (bl1 + br1).reshape(1, HC)
    mb2 = (bl2 + br2).reshape(1, HC)

    shared = {
        "xT": trbf(x_perm),
        "wlT1": trbf(Wl1), "wrT1": trbf(inputs["Wr1"]),
        "wlT2": trbf(inputs["Wl2"]), "wrT2": trbf(inputs["Wr2"]),
        "att1r": bf(np.tile(att1, (P, Gmax))),
        "att2r": bf(np.tile(att2, (P, Gmax))),
        "iotar": bf(np.tile(np.arange(P, dtype=np.float32)[None, :], (P, Gmax))),
        "bias1r": np.tile(b1eff.reshape(1, C), (P, 1)).astype(np.float32),
        "bias2r": np.tile(b2eff.reshape(1, C), (P, 1)).astype(np.float32),
        "blr1": mb1, "blr2": mb2,
        "onesr": np.ones((1, P), np.float32),
        "abT": bf(np.concatenate([Wd1[:, :C].T, Wd1[:, C:].T], axis=1)),
        "bd1r": np.concatenate([bd1.reshape(1, C),
                                np.zeros((1, C), np.float32)], axis=1),
        "wd2r": bf(np.tile(Wd2.reshape(1, C), (P, GD))),
        "iotac": np.arange(P, dtype=np.float32).reshape(P, 1),
    }

    in_maps = []
    for k in range(n_cores):
        tiles = slice(k * T, (k + 1) * T)
        m = dict(shared)
        m["srcI"] = np.ascontiguousarray(
            src_pad[tiles].reshape(T * S_sub, P).T)
        m["dstloc"] = bf(np.ascontiguousarray(
            dstloc_pad[tiles].reshape(T * S_sub, P).T))
        m["dstI"] = np.ascontiguousarray(
            dstg_pad[tiles].reshape(T * S_sub, P).T)
        es = slice(k * E_dec, (k + 1) * E_dec)
        m["selfI"] = np.ascontiguousarray(
            (k * NPC + np.arange(T)[None, :] * P
             + np.arange(P)[:, None]).astype(np.int32))
        m["decS"] = np.ascontiguousarray(
            src[es].astype(np.int32).reshape(DSUB, P).T)
        m["decD"] = np.ascontiguousarray(
            dst[es].astype(np.int32).reshape(DSUB, P).T)
        in_maps.append(m)

    cfg = {
        "N": N, "C": C, "H": H, "n_cores": n_cores, "S_sub": S_sub,
        "G": G, "GD": GD, "DSUB": DSUB,
        "has_mbias1": bool(np.abs(mb1).max() > 0),
        "has_mbias2": bool(np.abs(mb2).max() > 0),
        "has_bd1": bool(np.abs(bd1).max() > 0),
        "bd2": float(np.asarray(inputs["bd2"]).reshape(-1)[0]),
    }
    return in_maps, cfg


def postprocess(results, cfg):
    n_cores = cfg["n_cores"]
    out = np.concatenate(
        [results[k]["out_dec"].reshape(-1) for k in range(n_cores)])
    return (out + cfg["bd2"]).astype(np.float32)


def kernel(**inputs):
    from concourse.bass_utils import run_bass_kernel_spmd

    n_cores = 8
    in_maps, cfg = host_prep(inputs, n_cores=n_cores)
    nc = build_program(cfg)
    res = run_bass_kernel_spmd(nc, in_maps, list(range(n_cores)))
    return postprocess(res.results, cfg)
